# revision 11
# baseline (speedup 1.0000x reference)
# CARAFE (content-aware reassembly) Trainium2 Bass kernel.
# Strategy: data-parallel over batch (8 items -> 8 NeuronCores). Per core:
#   - 1x1 compressor conv (PE matmul) + folded BN + SiLU (ACT)
#   - 3x3 encoder conv as 9 accumulating matmuls on padded t (PE) + folded BN
#   - exp + per-class (2x2 subpixel) normalization for the 25-way softmax
#   - 25-tap reassembly as per-pixel fused multiply-accumulate on DVE
#     (scalar_tensor_tensor), with partition-shifted x windows produced by
#     shift-matrix matmuls on PE and evicted by ACT
#   - output re-transposed to channel-major by PE and DMA'd out.
import sys
import numpy as np

for _p in ("/opt/trn_rl_repo",):
    if _p not in sys.path:
        sys.path.insert(0, _p)

B, C, Cm, E = 8, 192, 64, 100
H = W = 64
K, S = 5, 2
EPS = 1e-3
NT = 32            # pixel tiles (2 rows x 64 cols = 128 pixels each)
NB = 36            # xT_v row blocks of 128 (rows r in [-4, 68))

# shift-matrix offsets tau: S_tau[k, m] = 1 iff k == m + tau
TAUS = sorted({0, 1, 2, 62, 63, 64, 65, 66, 126, 127,
               -1, -2, -62, -63, -64, -65, -66, -126, -127})
TAU_IDX = {t: i for i, t in enumerate(TAUS)}

_prog_cache = {}


def _build_program(num_devices=8):
    import concourse.mybir as mybir
    import concourse.tile as tile
    from concourse import bacc
    from contextlib import ExitStack

    fp32 = mybir.dt.float32
    AL = mybir.AluOpType
    AF = mybir.ActivationFunctionType

    nc = bacc.Bacc("TRN2", target_bir_lowering=False, num_devices=num_devices)

    x_d = nc.dram_tensor("x", [C, H * W], fp32, kind="ExternalInput").ap()
    cw_d = nc.dram_tensor("cw", [C, Cm], fp32, kind="ExternalInput").ap()
    cb_d = nc.dram_tensor("cb", [Cm, 1], fp32, kind="ExternalInput").ap()
    ew_d = nc.dram_tensor("ew", [Cm, 9 * E], fp32, kind="ExternalInput").ap()
    eb_d = nc.dram_tensor("eb", [E, 1], fp32, kind="ExternalInput").ap()
    edge_d = nc.dram_tensor("edge", [128, K * K], fp32, kind="ExternalInput").ap()
    shm_d = nc.dram_tensor("shm", [128, len(TAUS) * 128], fp32, kind="ExternalInput").ap()
    out_d = nc.dram_tensor("out", [C, H, S, S * W], fp32, kind="ExternalOutput").ap()

    es = ExitStack()
    with tile.TileContext(nc) as tc:
        with es:
            _body(es, tc, nc, mybir, fp32, AL, AF,
                  x_d, cw_d, cb_d, ew_d, eb_d, edge_d, shm_d, out_d)
    nc.compile()
    return nc


def _body(es, tc, nc, mybir, fp32, AL, AF,
          x_d, cw_d, cb_d, ew_d, eb_d, edge_d, shm_d, out_d):
    consts = es.enter_context(tc.tile_pool(name="consts", bufs=1))
    big = es.enter_context(tc.tile_pool(name="big", bufs=1))

    cw0 = consts.tile([128, Cm], fp32, tag="cw0")
    cw1 = consts.tile([64, Cm], fp32, tag="cw1")
    cb = consts.tile([Cm, 1], fp32, tag="cb")
    ew = consts.tile([Cm, 9 * E], fp32, tag="ew")
    eb = consts.tile([E, 1], fp32, tag="eb")
    edge = consts.tile([128, K * K], fp32, tag="edge")
    shm = consts.tile([128, len(TAUS), 128], fp32, tag="shm")

    x0 = big.tile([128, H * W], fp32, tag="x0")
    x1 = big.tile([64, H * W], fp32, tag="x1")
    t_pad = big.tile([Cm, 66 * 66], fp32, tag="tpad")
    e_sb = big.tile([E, H * W], fp32, tag="esb")
    xT = big.tile([128, NB, C], fp32, tag="xT")
    maskT = big.tile([128, NT, E], fp32, tag="maskT")
    rsum = big.tile([128, NT, 4], fp32, tag="rsum")

    nc.sync.dma_start(out=cw0[:], in_=cw_d[0:128, :])
    nc.sync.dma_start(out=cw1[:], in_=cw_d[128:192, :])
    nc.sync.dma_start(out=cb[:], in_=cb_d)
    nc.sync.dma_start(out=ew[:], in_=ew_d)
    nc.sync.dma_start(out=eb[:], in_=eb_d)
    nc.sync.dma_start(out=edge[:], in_=edge_d)
    nc.sync.dma_start(out=shm[:].rearrange("p a b -> p (a b)"), in_=shm_d)
    nc.sync.dma_start(out=x0[:], in_=x_d[0:128, :])
    nc.sync.dma_start(out=x1[:], in_=x_d[128:192, :])

    ident = shm[:, TAU_IDX[0], :]  # [128, 128] identity

    # zero borders of t_pad and the vertical zero blocks of xT
    nc.gpsimd.memset(t_pad[:], 0.0)
    nc.gpsimd.memset(xT[:, 0:2, :], 0.0)
    nc.gpsimd.memset(xT[:, NB - 2:NB, :], 0.0)

    # ---- transpose x into xT (row blocks offset by +256 rows of zero pad) ----
    with tc.tile_pool(name="tp_ps", bufs=4, space="PSUM") as tp_ps:
        for pb in range(32):  # pixel blocks of 128
            p0 = pb * 128
            q = pb + 2
            pt0 = tp_ps.tile([128, 128], fp32, tag="pt0")
            nc.tensor.transpose(pt0[:], x0[:, p0:p0 + 128], ident)
            nc.scalar.copy(out=xT[:, q, 0:128], in_=pt0[:])
            pt1 = tp_ps.tile([128, 64], fp32, tag="pt1")
            nc.tensor.transpose(pt1[:], x1[:, p0:p0 + 128], ident[0:64, 0:64])
            nc.scalar.copy(out=xT[:, q, 128:192], in_=pt1[:])

    # ---- compressor 1x1 conv + SiLU -> t_pad interior ----
    with tc.tile_pool(name="c1_ps", bufs=2, space="PSUM") as c1_ps, \
         tc.tile_pool(name="c1sg", bufs=2) as c1sg:
        for nt in range(8):  # 8 tiles of 512 pixels = 8 rows
            n0 = nt * 512
            ps = c1_ps.tile([Cm, 512], fp32, tag="c1")
            nc.tensor.matmul(ps[:], cw0[:], x0[:, n0:n0 + 512], start=True, stop=False)
            nc.tensor.matmul(ps[:], cw1[:], x1[:, n0:n0 + 512], start=False, stop=True)
            # write into padded t at rows nt*8 .. nt*8+8, offset (+1,+1)
            # silu(y) = y*sigmoid(y) with y = ps + cb
            sg = c1sg.tile([Cm, 512], fp32, tag="sg")
            nc.scalar.activation(out=sg[:], in_=ps[:], func=AF.Sigmoid, bias=cb[:], scale=1.0)
            v = t_pad[:].rearrange("c (r z) -> c r z", z=66)[:, nt * 8 + 1: nt * 8 + 9, 1:65]
            nc.vector.scalar_tensor_tensor(
                v, ps[:].rearrange("c (r z) -> c r z", z=64), cb[:],
                sg[:].rearrange("c (r z) -> c r z", z=64), AL.add, AL.mult)

    # ---- encoder 3x3 conv + exp ----
    with tc.tile_pool(name="c2_ps", bufs=2, space="PSUM") as c2_ps:
        for nt in range(8):
            r0 = nt * 8
            ps = c2_ps.tile([E, 512], fp32, tag="c2")
            for tap in range(9):
                dy, dx = tap // 3, tap % 3
                rhs = t_pad[:].rearrange("c (r z) -> c r z", z=66)[:, r0 + dy: r0 + dy + 8, dx: dx + 64]
                nc.tensor.matmul(ps[:], ew[:, tap * E:(tap + 1) * E], rhs,
                                 start=(tap == 0), stop=(tap == 8))
            nc.scalar.activation(out=e_sb[:, nt * 512:(nt + 1) * 512], in_=ps[:],
                                 func=AF.Exp, bias=eb[:], scale=1.0)

    # ---- transpose mask to pixel-major + normalize ----
    with tc.tile_pool(name="mt_ps", bufs=4, space="PSUM") as mt_ps:
        for ti in range(NT):
            p0 = ti * 128
            pt = mt_ps.tile([128, E], fp32, tag="mt")
            nc.tensor.transpose(pt[:], e_sb[:, p0:p0 + 128], ident[0:E, 0:E])
            nc.scalar.copy(out=maskT[:, ti, :], in_=pt[:])
            # maskT free layout: ch = ij*4 + cl
            v_cl_ij = maskT[:, ti, :].rearrange("p (ij cl) -> p cl ij", cl=4)
            s = rsum[:, ti, :]
            nc.vector.tensor_reduce(out=s, in_=v_cl_ij, axis=mybir.AxisListType.X, op=AL.add)
            nc.vector.reciprocal(s, s)
            e_cl_ij = edge[:].unsqueeze(1).broadcast_to([128, 4, K * K])
            nc.vector.tensor_tensor(v_cl_ij, v_cl_ij, e_cl_ij, AL.mult)
            v_ij_cl = maskT[:, ti, :].rearrange("p (ij cl) -> p ij cl", cl=4)
            r_b = rsum[:, ti, :].unsqueeze(1).broadcast_to([128, K * K, 4])
            nc.vector.tensor_tensor(v_ij_cl, v_ij_cl, r_b, AL.mult)

    # ---- main reassembly ----
    win_ps = es.enter_context(tc.tile_pool(name="win_ps", bufs=3, space="PSUM"))
    out_ps = es.enter_context(tc.tile_pool(name="out_ps", bufs=4, space="PSUM"))
    work = es.enter_context(tc.tile_pool(name="work", bufs=40))
    accp = es.enter_context(tc.tile_pool(name="accp", bufs=8))
    stagep = es.enter_context(tc.tile_pool(name="stagep", bufs=3))

    win_cache = {}
    for ti in range(NT):
        h0 = ti * 2
        wins = []
        for i in range(K):
            for j in range(K):
                key = (h0 + i + 2, j - 2)
                if key not in win_cache:
                    R0 = key[0] * 64 + key[1]
                    q, sig = R0 // 128, R0 % 128
                    ps = win_ps.tile([128, C], fp32, tag="winps")
                    nc.tensor.matmul(ps[:], shm[:, TAU_IDX[sig], :], xT[:, q, :],
                                     start=True, stop=(sig == 0))
                    if sig != 0:
                        nc.tensor.matmul(ps[:], shm[:, TAU_IDX[sig - 128], :], xT[:, q + 1, :],
                                         start=False, stop=True)
                    w_sb = work.tile([128, C], fp32, tag="win")
                    nc.scalar.copy(out=w_sb[:], in_=ps[:])
                    win_cache[key] = w_sb
                wins.append(win_cache[key])
        # retire windows no longer needed (keep pool pressure bounded)
        win_cache = {k: v for k, v in win_cache.items() if k[0] >= h0 + 3}
        stg_tiles = {(di, ch): stagep.tile([96, 2, S * W], fp32, name=f"stg{di}_{ch}", tag=f"stg{di}_{ch}")
                     for di in range(2) for ch in range(2)}
        for cl in range(4):
            acc = accp.tile([128, C], fp32, tag="acc")
            for ij in range(K * K):
                col = maskT[:, ti, ij * 4 + cl:ij * 4 + cl + 1]
                if ij == 0:
                    nc.vector.tensor_scalar(acc[:], wins[ij][:], col, None, AL.mult)
                else:
                    nc.vector.scalar_tensor_tensor(acc[:], wins[ij][:], col, acc[:],
                                                   AL.mult, AL.add)
            # transpose acc -> [c, pix] and stage
            di, dj = cl // 2, cl % 2
            for ch in range(2):
                c0 = ch * 96
                pt = out_ps.tile([96, 128], fp32, tag="ot")
                nc.tensor.transpose(pt[:], acc[:, c0:c0 + 96], ident)
                stg = stg_tiles[(di, ch)]
                dst = stg[:].rearrange("c h (w t) -> c h w t", t=2)[:, :, :, dj]
                nc.scalar.copy(out=dst, in_=pt[:].rearrange("c (h w) -> c h w", h=2))
                if dj == 1:
                    nc.sync.dma_start(
                        out=out_d[c0:c0 + 96, h0:h0 + 2, di, :],
                        in_=stg[:])
    es.pop_all().close()


def _host_prep(inputs):
    def fold(w, g, b, m, v):
        s = g / np.sqrt(v + EPS)
        return (w * s[:, None, None, None]).astype(np.float32), (b - m * s).astype(np.float32)

    comp_w_eff, comp_b_eff = fold(inputs["comp_w"], inputs["comp_g"], inputs["comp_b"],
                                  inputs["comp_m"], inputs["comp_v"])
    enc_w_eff, enc_b_eff = fold(inputs["enc_w"], inputs["enc_g"], inputs["enc_b"],
                                inputs["enc_m"], inputs["enc_v"])
    cw = np.ascontiguousarray(comp_w_eff[:, :, 0, 0].T)          # [192, 64]
    cb = comp_b_eff.reshape(Cm, 1)
    ew = np.concatenate([enc_w_eff[:, :, dy, dx].T
                         for dy in range(3) for dx in range(3)], axis=1)  # [64, 900]
    ew = np.ascontiguousarray(ew)
    eb = enc_b_eff.reshape(E, 1)
    wv = np.arange(128) % 64
    edge = np.zeros((128, K * K), np.float32)
    for j in range(K):
        ok = (wv + j - 2 >= 0) & (wv + j - 2 < W)
        for i in range(K):
            edge[:, i * K + j] = ok
    shm = np.zeros((128, len(TAUS), 128), np.float32)
    for t, i in TAU_IDX.items():
        shm[:, i, :] = np.eye(128, dtype=np.float32, k=-t)
    shm = shm.reshape(128, len(TAUS) * 128)
    return dict(cw=cw, cb=cb, ew=ew, eb=eb, edge=edge, shm=shm)


def kernel(**inputs):
    from concourse.bass_utils import run_bass_kernel_spmd

    inputs = {k: np.asarray(v, dtype=np.float32) for k, v in inputs.items()}
    w = _host_prep(inputs)
    if "nc" not in _prog_cache:
        _prog_cache["nc"] = _build_program()
    nc = _prog_cache["nc"]
    x = inputs["x"]
    in_maps = [dict(x=np.ascontiguousarray(x[b].reshape(C, H * W)), **w) for b in range(B)]
    res = run_bass_kernel_spmd(nc, in_maps, list(range(B)))
    out = np.stack([res.results[b]["out"].reshape(C, 2 * H, 2 * W) for b in range(B)])
    return out
